# revision 8
# baseline (speedup 1.0000x reference)
"""Trainium2 Bass kernel for nn_Net_67954972557347 (dense_mlp).

Network: a1 = lrelu(a@Wa+ba) [B,68]; b1 = lrelu(b@Wb+bb) [B,68];
c = [a1|b1|meta] [B,140]; then 10 lrelu'd dense layers
(140->34->34->20->20->20->20->20->5->2->1), lrelu slope 0.01.

Strategy: pure data parallel over 8 cores (32768 rows each), activations
feature-major ([feat, batch]), batch streamed 512 columns per chunk.
All tensors fp16 (same 10-bit mantissa as tf32; fp32 PSUM accumulation),
which runs the PE at 1 cycle/row with fast weight load.

The 12 layers are packed into FOUR matmuls per chunk via a software
pipeline (each chunk advances one stage per step; stages of different
chunks colocate in one moving tile / one stationary matrix):

  TA [122p] = [c0(34); c2(20); c6(20); c8(2); a(45); ones]
    --G1--> Q1 [a1(68); c1(34); c3(20); c7(5); y(1)] = 128 cols
  TB [123p] = [c4(20); b(102); ones]
    --G2--> Q2 [b1(68); c5(20)]
  TC [128p] = [a1; c1; c3; c7; y] (drained Q1)
    --G3--> Q34 [c0p; c2; c8; c4@96] (start)
  TD [93p]  = [b1; c5; meta(4); ones] (drained Q2 + meta DMA)
    --G4--> Q34 [c0p; c6]            (accum, stop)

Q34 col layout [c0 0:34; c2 34:54; c6 54:74; c8 74:76; pad; c4 96:116]
is padded so every drain window obeys the engine partition-window rule
(start in {0,32,64,96}, count within the aligned block):

  Q1[0:128]   -> TC'[0:128]  (ScalarE Prelu)
  Q34[0:76]   -> TA'[0:76]   (ScalarE Prelu)
  Q2[0:88]    -> TD'[0:88]   (VectorE cast + stt lrelu)
  Q34[96:116] -> TB'[0:20]   (VectorE cast + stt lrelu)

All biases ride the ones rows (bias rows of the stationaries accumulate
into PSUM), so drains are pure leaky-relu. y is DMA'd from TC row 127.
"""

import os
import sys

import numpy as np

for _p in ("/opt/trn_rl_repo", "/root/.axon_site/_ro/trn_rl_repo"):
    if os.path.isdir(_p) and _p not in sys.path:
        sys.path.append(_p)

import concourse.bass as bass
import concourse.mybir as mybir
import concourse.tile as tile
from concourse import bacc
from concourse.bass_utils import run_bass_kernel_spmd
from bass_rust import add_dep_helper

F16 = mybir.dt.float16
F32 = mybir.dt.float32
ALU = mybir.AluOpType
PRELU = mybir.ActivationFunctionType.Prelu

B_FULL = 262144
N_CORES = 8
B_CORE = B_FULL // N_CORES          # 32768
N = 512                              # columns per chunk (PSUM fp32 cap)
PIPE = 10                            # pipeline depth in steps
ALPHA = 0.01

# TA partition layout
TA_C0, TA_C2, TA_C6, TA_C8, TA_CE, TA_A, TA_ONE, TA_END = \
    0, 34, 54, 74, 76, 76, 121, 122
# TB layout
TB_C4, TB_B, TB_ONE, TB_END = 0, 20, 122, 123
# TC layout (== Q1 column layout)
TC_A1, TC_C1, TC_C3, TC_C7, TC_Y, TC_END = 0, 68, 102, 122, 127, 128
# TD layout
TD_B1, TD_C5, TD_M, TD_ONE, TD_END = 0, 68, 88, 92, 93
# Q2 column layout
Q2_B1, Q2_C5, Q2_END = 0, 68, 88
# Q34 column layout (pad 76:96 so the c4 window starts at partition 96)
Q34_C0, Q34_C2, Q34_C6, Q34_C8, Q34_CE, Q34_C4, Q34_C4E = \
    0, 34, 54, 74, 76, 96, 116


def _pack_weights(inp):
    """Four [128,128] fp16 stationaries packed into one [128, 512] tile."""
    f32 = lambda k: np.asarray(inp[k], np.float32)
    Wa, ba, Wb, bb = f32("Wa"), f32("ba"), f32("Wb"), f32("bb")
    W = [f32(f"W{i}") for i in range(10)]
    B = [f32(f"B{i}") for i in range(10)]
    wt = np.zeros((128, 512), np.float32)

    g1 = wt[:, 0:128]
    g1[TA_C0:TA_C2, TC_C1:TC_C3] = W[1]
    g1[TA_C2:TA_C6, TC_C3:TC_C7] = W[3]
    g1[TA_C6:TA_C8, TC_C7:TC_Y] = W[7]
    g1[TA_C8:TA_CE, TC_Y:TC_END] = W[9]
    g1[TA_A:TA_ONE, TC_A1:TC_C1] = Wa
    g1[TA_ONE, TC_A1:TC_C1] = ba
    g1[TA_ONE, TC_C1:TC_C3] = B[1]
    g1[TA_ONE, TC_C3:TC_C7] = B[3]
    g1[TA_ONE, TC_C7:TC_Y] = B[7]
    g1[TA_ONE, TC_Y:TC_END] = B[9]

    g2 = wt[:, 128:256]
    g2[TB_C4:TB_B, Q2_C5:Q2_END] = W[5]
    g2[TB_B:TB_ONE, Q2_B1:Q2_C5] = Wb
    g2[TB_ONE, Q2_B1:Q2_C5] = bb
    g2[TB_ONE, Q2_C5:Q2_END] = B[5]

    g3 = wt[:, 256:384]
    g3[TC_A1:TC_C1, Q34_C0:Q34_C2] = W[0][0:68]
    g3[TC_C1:TC_C3, Q34_C2:Q34_C6] = W[2]
    g3[TC_C3:TC_C7, Q34_C4:Q34_C4E] = W[4]
    g3[TC_C7:TC_Y, Q34_C8:Q34_CE] = W[8]

    g4 = wt[:, 384:512]
    g4[TD_B1:TD_C5, Q34_C0:Q34_C2] = W[0][68:136]
    g4[TD_C5:TD_M, Q34_C6:Q34_C8] = W[6]
    g4[TD_M:TD_ONE, Q34_C0:Q34_C2] = W[0][136:140]
    g4[TD_ONE, Q34_C0:Q34_C2] = B[0]
    g4[TD_ONE, Q34_C2:Q34_C6] = B[2]
    g4[TD_ONE, Q34_C6:Q34_C8] = B[6]
    g4[TD_ONE, Q34_C8:Q34_CE] = B[8]
    g4[TD_ONE, Q34_C4:Q34_C4E] = B[4]

    return wt.astype(np.float16)


def _pack_core_inputs(a, b, meta):
    """One core's shard -> (tA [46,Bc], tB [103,Bc], tM [5,Bc]) fp16."""
    bc = a.shape[0]
    tA = np.empty((46, bc), np.float16)
    tA[0:45] = a.astype(np.float16).T
    tA[45] = 1.0
    tB = np.empty((103, bc), np.float16)
    tB[0:102] = b.astype(np.float16).T
    tB[102] = 1.0
    tM = np.empty((5, bc), np.float16)
    tM[0:4] = meta.astype(np.float16).T
    tM[4] = 1.0
    return tA, tB, tM


def build_bass(n_chunks):
    nc = bacc.Bacc(None, target_bir_lowering=False, debug=False)
    n_steps = n_chunks + PIPE          # 74

    tA_d = nc.dram_tensor("tA", [46, n_chunks * N], F16, kind="ExternalInput")
    tB_d = nc.dram_tensor("tB", [103, n_chunks * N], F16, kind="ExternalInput")
    tM_d = nc.dram_tensor("tM", [5, n_chunks * N], F16, kind="ExternalInput")
    wt_d = nc.dram_tensor("wt", [128, 512], F16, kind="ExternalInput")
    y_d = nc.dram_tensor("y", [1, n_chunks * N], F16, kind="ExternalOutput")

    with tile.TileContext(nc) as tc:
        with (
            tc.tile_pool(name="const", bufs=1) as constp,
            tc.tile_pool(name="tap", bufs=4) as tap,
            tc.tile_pool(name="tbp", bufs=4) as tbp,
            tc.tile_pool(name="tcp", bufs=3) as tcp,
            tc.tile_pool(name="tdp", bufs=3) as tdp,
            tc.tile_pool(name="ps", bufs=2, space=bass.MemorySpace.PSUM) as ps,
        ):
            wt = constp.tile([128, 512], F16, tag="wt")
            nc.sync.dma_start(wt[:], wt_d[:])
            wg1, wg2 = wt[:, 0:128], wt[:, 128:256]
            wg3, wg4 = wt[:, 256:384], wt[:, 384:512]

            def chain(*insts):
                for i in range(1, len(insts)):
                    add_dep_helper(insts[i].ins, insts[i - 1].ins,
                                   sync=False, reason="psum acc order")

            TA, TB, TCt, TD = {}, {}, {}, {}

            def new_tiles(t):
                TA[t] = tap.tile([128, N], F16, tag="ta", name=f"ta_{t}")
                TB[t] = tbp.tile([128, N], F16, tag="tb", name=f"tb_{t}")
                TCt[t] = tcp.tile([128, N], F16, tag="tc", name=f"tc_{t}")
                TD[t] = tdp.tile([128, N], F16, tag="td", name=f"td_{t}")
                if t == 0:
                    for x in (TA[t], TB[t], TCt[t], TD[t]):
                        nc.gpsimd.memset(x[0:128], 0.0)
                if t < n_chunks:
                    cs = slice(t * N, (t + 1) * N)
                    nc.sync.dma_start(TA[t][TA_A:TA_END], tA_d[:, cs])
                    nc.sync.dma_start(TB[t][TB_B:TB_END], tB_d[:, cs])
                else:
                    # tail: re-DMA chunk 0 (ones rows live; rest discarded)
                    nc.sync.dma_start(TA[t][TA_A:TA_END], tA_d[:, 0:N])
                    nc.sync.dma_start(TB[t][TB_B:TB_END], tB_d[:, 0:N])
                # meta lags one step: G4 at step t consumes chunk t-1's meta
                if 1 <= t <= n_chunks:
                    cs = slice((t - 1) * N, t * N)
                    nc.sync.dma_start(TD[t][TD_M:TD_END], tM_d[:, cs])
                else:
                    nc.sync.dma_start(TD[t][TD_M:TD_END], tM_d[:, 0:N])

            new_tiles(0)

            mm = nc.tensor.matmul
            for t in range(n_steps):
                new_tiles(t + 1)
                q1 = ps.tile([128, N], F32, tag="q1", name=f"q1_{t}")
                mm(q1[0:128], wg1[0:TA_END], TA[t][0:TA_END],
                   start=True, stop=True, tile_position=(0, 0))
                q2 = ps.tile([128, N], F32, tag="q2", name=f"q2_{t}")
                mm(q2[0:128], wg2[0:TB_END], TB[t][0:TB_END],
                   start=True, stop=True, tile_position=(0, 0))
                q34 = ps.tile([128, N], F32, tag="q34", name=f"q34_{t}")
                i3 = mm(q34[0:128], wg3[0:TC_END], TCt[t][0:TC_END],
                        start=True, stop=False, tile_position=(0, 0))
                i4 = mm(q34[0:128], wg4[0:TD_END], TD[t][0:TD_END],
                        start=False, stop=True, tile_position=(0, 0))
                chain(i3, i4)

                # ---- drains into step t+1 tiles ----
                nc.scalar.activation(TCt[t + 1][0:TC_END], q1[0:TC_END],
                                     PRELU, alpha=ALPHA)
                nc.scalar.activation(TA[t + 1][TA_C0:TA_CE],
                                     q34[Q34_C0:Q34_CE], PRELU, alpha=ALPHA)
                nc.vector.tensor_copy(TB[t + 1][TB_C4:TB_B],
                                      q34[Q34_C4:Q34_C4E])
                nc.vector.scalar_tensor_tensor(
                    TB[t + 1][TB_C4:TB_B], TB[t + 1][TB_C4:TB_B],
                    ALPHA, TB[t + 1][TB_C4:TB_B], ALU.mult, ALU.max)
                nc.vector.tensor_copy(TD[t + 1][0:Q2_END], q2[0:Q2_END])
                nc.vector.scalar_tensor_tensor(
                    TD[t + 1][0:Q2_END], TD[t + 1][0:Q2_END],
                    ALPHA, TD[t + 1][0:Q2_END], ALU.mult, ALU.max)

                # ---- y out (chunk t-10 sits in TC[t+1] row 127) ----
                if t >= PIPE:
                    c = t - PIPE
                    nc.gpsimd.dma_start(y_d[:, c * N:(c + 1) * N],
                                        TCt[t + 1][TC_Y:TC_END])

    nc.compile()
    return nc


_NC_CACHE = {}


def _get_nc(n_chunks):
    if n_chunks not in _NC_CACHE:
        _NC_CACHE[n_chunks] = build_bass(n_chunks)
    return _NC_CACHE[n_chunks]


def run_cores(inputs, n_chunks, cores, trace=False, trace_kwargs=None):
    a = np.asarray(inputs["a"], np.float32)
    b = np.asarray(inputs["b"], np.float32)
    meta = np.asarray(inputs["meta"], np.float32)
    wt = _pack_weights(inputs)
    in_maps = []
    for r in cores:
        sl = slice(r * B_CORE, r * B_CORE + n_chunks * N)
        tA, tB, tM = _pack_core_inputs(a[sl], b[sl], meta[sl])
        in_maps.append({"tA": tA, "tB": tB, "tM": tM, "wt": wt})
    nc = _get_nc(n_chunks)
    kw = dict(trace=trace)
    if trace_kwargs:
        kw.update(trace_kwargs)
    res = run_bass_kernel_spmd(nc, in_maps, list(range(len(cores))), **kw)
    return [res.results[i]["y"] for i in range(len(cores))], res


def kernel(**inputs):
    n_chunks = B_CORE // N
    ys, _ = run_cores(inputs, n_chunks, list(range(N_CORES)))
    out = np.empty((B_FULL, 1), np.float32)
    for r in range(N_CORES):
        out[r * B_CORE:(r + 1) * B_CORE, 0] = ys[r][0].astype(np.float32)
    return out


# revision 10
# speedup vs baseline: 1.0271x; 1.0271x over previous
"""Trainium2 Bass kernel for nn_Net_67954972557347 (dense_mlp).

Network: a1 = lrelu(a@Wa+ba) [B,68]; b1 = lrelu(b@Wb+bb) [B,68];
c = [a1|b1|meta] [B,140]; then 10 lrelu'd dense layers
(140->34->34->20->20->20->20->20->5->2->1), lrelu slope 0.01.

Strategy: pure data parallel over 8 cores (32768 rows each), activations
feature-major ([feat, batch]), batch streamed 512 columns per chunk.
All tensors fp16 (same 10-bit mantissa as tf32; fp32 PSUM accumulation):
PE runs at 1 cycle/row with fast weight load.

The 12 layers are packed into FOUR matmuls per chunk via a software
pipeline (one layer-hop per step; stages of different chunks colocate in
one moving tile / one stationary matrix):

  TA [122p] = [c0(34); c2(20); c6(20); c8(2); a(45); ones]     (quad tile)
    --G1--> Q12 bank0: [a1(68); c1(34); c3(20); c7(5); y(1)] = 128
  TB [127p] = [c4(20); b(102); ones; ilrelu(meta)(4)]          (quad tile)
    --G2--> Q12 bank1: [b1(68); c5(20); meta(4); one(1)]  (passthroughs!)
  U[:, 0:512] = TC = Q12-bank0 drained; U[:, 512:1024] = TD = bank1 drained
  U-TC [128p] --G3--> Q34 [c0p; c2; -; c8; pad; c4@96]   (start)
  U-TD [93p]  --G4--> Q34 [c0p; -; c6; -; -]             (accum, stop)

Drains (3 per chunk; partition windows obey the 0/32/64/96 rule):
  Q12[0:128, 0:1024] -> U'[0:128, 0:1024]   ONE ScalarE Prelu (2 banks)
  Q34[0:76]          -> TA'[0:76]           ScalarE Prelu
  Q34[96:116]        -> TB'[0:20]           VectorE cast + stt lrelu

meta and the ones row ride G2 as identity passthrough columns (host
pre-applies inverse-lrelu so the Prelu drain recovers them); all biases
ride the ones rows, so drains are pure leaky-relu. Inputs are DMA'd in
4-chunk quads (2 big DMAs per 4 steps); y is DMA'd from U row 127.
"""

import os
import sys

import numpy as np

for _p in ("/opt/trn_rl_repo", "/root/.axon_site/_ro/trn_rl_repo"):
    if os.path.isdir(_p) and _p not in sys.path:
        sys.path.append(_p)

import concourse.bass as bass
import concourse.mybir as mybir
import concourse.tile as tile
from concourse import bacc
from concourse.bass_utils import run_bass_kernel_spmd
from bass_rust import add_dep_helper

F16 = mybir.dt.float16
F32 = mybir.dt.float32
ALU = mybir.AluOpType
PRELU = mybir.ActivationFunctionType.Prelu

B_FULL = 262144
N_CORES = 8
B_CORE = B_FULL // N_CORES          # 32768
N = 512                              # columns per chunk (PSUM fp32 cap)
PIPE = 10                            # pipeline depth in steps
ALPHA = 0.01
QUAD = 4                             # chunks per input tile / DMA

# TA partition layout
TA_C0, TA_C2, TA_C6, TA_C8, TA_CE, TA_A, TA_ONE, TA_END = \
    0, 34, 54, 74, 76, 76, 121, 122
# TB layout
TB_C4, TB_B, TB_ONE, TB_M, TB_END = 0, 20, 122, 123, 127
# U cols 0:512 = TC rows (== Q12 bank0 cols)
TC_A1, TC_C1, TC_C3, TC_C7, TC_Y, TC_END = 0, 68, 102, 122, 127, 128
# U cols 512:1024 = TD rows (== Q12 bank1 cols)
TD_B1, TD_C5, TD_M, TD_ONE, TD_END = 0, 68, 88, 92, 93
# Q34 column layout (pad 76:96 so the c4 window starts at partition 96)
Q34_C0, Q34_C2, Q34_C6, Q34_C8, Q34_CE, Q34_C4, Q34_C4E = \
    0, 34, 54, 74, 76, 96, 116


def _pack_weights(inp):
    """Four [128,128] fp16 stationaries packed into one [128, 512] tile."""
    f32 = lambda k: np.asarray(inp[k], np.float32)
    Wa, ba, Wb, bb = f32("Wa"), f32("ba"), f32("Wb"), f32("bb")
    W = [f32(f"W{i}") for i in range(10)]
    B = [f32(f"B{i}") for i in range(10)]
    wt = np.zeros((128, 512), np.float32)

    g1 = wt[:, 0:128]
    g1[TA_C0:TA_C2, TC_C1:TC_C3] = W[1]
    g1[TA_C2:TA_C6, TC_C3:TC_C7] = W[3]
    g1[TA_C6:TA_C8, TC_C7:TC_Y] = W[7]
    g1[TA_C8:TA_CE, TC_Y:TC_END] = W[9]
    g1[TA_A:TA_ONE, TC_A1:TC_C1] = Wa
    g1[TA_ONE, TC_A1:TC_C1] = ba
    g1[TA_ONE, TC_C1:TC_C3] = B[1]
    g1[TA_ONE, TC_C3:TC_C7] = B[3]
    g1[TA_ONE, TC_C7:TC_Y] = B[7]
    g1[TA_ONE, TC_Y:TC_END] = B[9]

    g2 = wt[:, 128:256]
    g2[TB_C4:TB_B, TD_C5:TD_M] = W[5]
    g2[TB_B:TB_ONE, TD_B1:TD_C5] = Wb
    g2[TB_ONE, TD_B1:TD_C5] = bb
    g2[TB_ONE, TD_C5:TD_M] = B[5]
    g2[TB_ONE, TD_ONE] = 1.0                       # ones passthrough
    g2[TB_M:TB_END, TD_M:TD_ONE] = np.eye(4)       # meta passthrough

    g3 = wt[:, 256:384]
    g3[TC_A1:TC_C1, Q34_C0:Q34_C2] = W[0][0:68]
    g3[TC_C1:TC_C3, Q34_C2:Q34_C6] = W[2]
    g3[TC_C3:TC_C7, Q34_C4:Q34_C4E] = W[4]
    g3[TC_C7:TC_Y, Q34_C8:Q34_CE] = W[8]

    g4 = wt[:, 384:512]
    g4[TD_B1:TD_C5, Q34_C0:Q34_C2] = W[0][68:136]
    g4[TD_C5:TD_M, Q34_C6:Q34_C8] = W[6]
    g4[TD_M:TD_ONE, Q34_C0:Q34_C2] = W[0][136:140]
    g4[TD_ONE, Q34_C0:Q34_C2] = B[0]
    g4[TD_ONE, Q34_C2:Q34_C6] = B[2]
    g4[TD_ONE, Q34_C6:Q34_C8] = B[6]
    g4[TD_ONE, Q34_C8:Q34_CE] = B[8]
    g4[TD_ONE, Q34_C4:Q34_C4E] = B[4]

    return wt.astype(np.float16)


def _pack_core_inputs(a, b, meta, n_stream):
    """One core's shard -> (tA [46, n_stream*N], tB [107, n_stream*N]).

    Streams are padded past the shard with chunk-0 data so tail-step quad
    DMAs read valid memory (results discarded)."""
    bc = a.shape[0]
    ns = n_stream * N
    tA = np.empty((46, ns), np.float16)
    tA[0:45, :bc] = a.astype(np.float16).T
    tA[45] = 1.0
    tB = np.empty((107, ns), np.float16)
    tB[0:102, :bc] = b.astype(np.float16).T
    tB[102] = 1.0
    m = meta.astype(np.float32)
    tB[103:107, :bc] = np.where(m > 0, m, m * 100.0).astype(np.float16).T
    tA[0:45, bc:] = tA[0:45, 0:1]
    tB[0:102, bc:] = tB[0:102, 0:1]
    tB[103:107, bc:] = tB[103:107, 0:1]
    return tA, tB


def build_bass(n_chunks):
    nc = bacc.Bacc(None, target_bir_lowering=False, debug=False)
    n_steps = n_chunks + PIPE                       # 74
    n_quads = (n_steps + 1 + QUAD - 1) // QUAD      # tiles 0..n_steps
    n_stream = n_quads * QUAD

    tA_d = nc.dram_tensor("tA", [46, n_stream * N], F16, kind="ExternalInput")
    tB_d = nc.dram_tensor("tB", [107, n_stream * N], F16, kind="ExternalInput")
    wt_d = nc.dram_tensor("wt", [128, 512], F16, kind="ExternalInput")
    y_d = nc.dram_tensor("y", [1, n_chunks * N], F16, kind="ExternalOutput")

    with tile.TileContext(nc) as tc:
        with (
            tc.tile_pool(name="const", bufs=1) as constp,
            tc.tile_pool(name="tap", bufs=3) as tap,
            tc.tile_pool(name="tbp", bufs=3) as tbp,
            tc.tile_pool(name="up", bufs=3) as up,
            tc.tile_pool(name="ps", bufs=2, space=bass.MemorySpace.PSUM) as ps,
        ):
            wt = constp.tile([128, 512], F16, tag="wt")
            nc.sync.dma_start(wt[:], wt_d[:])
            wg1, wg2 = wt[:, 0:128], wt[:, 128:256]
            wg3, wg4 = wt[:, 256:384], wt[:, 384:512]

            def chain(*insts):
                for i in range(1, len(insts)):
                    add_dep_helper(insts[i].ins, insts[i - 1].ins,
                                   sync=False, reason="psum acc order")

            TAq, TBq, U = {}, {}, {}

            def new_quad(p):
                TAq[p] = tap.tile([128, QUAD * N], F16, tag="ta", name=f"ta_{p}")
                TBq[p] = tbp.tile([128, QUAD * N], F16, tag="tb", name=f"tb_{p}")
                if p == 0:
                    nc.gpsimd.memset(TAq[p][0:128], 0.0)
                    nc.gpsimd.memset(TBq[p][0:128], 0.0)
                qs = slice(p * QUAD * N, (p + 1) * QUAD * N)
                nc.sync.dma_start(TAq[p][TA_A:TA_END], tA_d[:, qs])
                nc.sync.dma_start(TBq[p][TB_B:TB_END], tB_d[:, qs])

            def new_u(t):
                U[t] = up.tile([128, 2 * N], F16, tag="u", name=f"u_{t}")
                if t == 0:
                    nc.gpsimd.memset(U[t][0:128], 0.0)

            new_quad(0)
            new_u(0)

            mm = nc.tensor.matmul
            for t in range(n_steps):
                if (t + 1) % QUAD == 0:
                    new_quad((t + 1) // QUAD)
                new_u(t + 1)
                ta = TAq[t // QUAD]
                tb = TBq[t // QUAD]
                cs = slice((t % QUAD) * N, (t % QUAD + 1) * N)
                ncs = slice(((t + 1) % QUAD) * N, ((t + 1) % QUAD + 1) * N)

                q12 = ps.tile([128, 2 * N], F32, tag="q12", name=f"q12_{t}")
                mm(q12[0:128, 0:N], wg1[0:TA_END], ta[0:TA_END, cs],
                   start=True, stop=True, tile_position=(0, 0))
                mm(q12[0:128, N:2 * N], wg2[0:TB_END], tb[0:TB_END, cs],
                   start=True, stop=True, tile_position=(0, 0))
                q34 = ps.tile([128, N], F32, tag="q34", name=f"q34_{t}")
                i3 = mm(q34[0:128], wg3[0:TC_END], U[t][0:TC_END, 0:N],
                        start=True, stop=False, tile_position=(0, 0))
                i4 = mm(q34[0:128], wg4[0:TD_END], U[t][0:TD_END, N:2 * N],
                        start=False, stop=True, tile_position=(0, 0))
                chain(i3, i4)

                # ---- drains into step t+1 tiles ----
                nc.scalar.activation(U[t + 1][0:128, 0:2 * N],
                                     q12[0:128, 0:2 * N], PRELU, alpha=ALPHA)
                nta = TAq[(t + 1) // QUAD]
                ntb = TBq[(t + 1) // QUAD]
                nc.scalar.activation(nta[TA_C0:TA_CE, ncs],
                                     q34[Q34_C0:Q34_CE], PRELU, alpha=ALPHA)
                nc.vector.tensor_copy(ntb[TB_C4:TB_B, ncs],
                                      q34[Q34_C4:Q34_C4E])
                nc.vector.scalar_tensor_tensor(
                    ntb[TB_C4:TB_B, ncs], ntb[TB_C4:TB_B, ncs],
                    ALPHA, ntb[TB_C4:TB_B, ncs], ALU.mult, ALU.max)

                # ---- y out (chunk t-10 sits in U[t+1] row 127, TC half) ----
                if t >= PIPE:
                    c = t - PIPE
                    nc.gpsimd.dma_start(y_d[:, c * N:(c + 1) * N],
                                        U[t + 1][TC_Y:TC_END, 0:N])

    nc.compile()
    return nc


_NC_CACHE = {}


def _get_nc(n_chunks):
    if n_chunks not in _NC_CACHE:
        _NC_CACHE[n_chunks] = build_bass(n_chunks)
    return _NC_CACHE[n_chunks]


def run_cores(inputs, n_chunks, cores, trace=False, trace_kwargs=None):
    a = np.asarray(inputs["a"], np.float32)
    b = np.asarray(inputs["b"], np.float32)
    meta = np.asarray(inputs["meta"], np.float32)
    wt = _pack_weights(inputs)
    n_steps = n_chunks + PIPE
    n_stream = ((n_steps + 1 + QUAD - 1) // QUAD) * QUAD
    in_maps = []
    for r in cores:
        sl = slice(r * B_CORE, r * B_CORE + n_chunks * N)
        tA, tB = _pack_core_inputs(a[sl], b[sl], meta[sl], n_stream)
        in_maps.append({"tA": tA, "tB": tB, "wt": wt})
    nc = _get_nc(n_chunks)
    kw = dict(trace=trace)
    if trace_kwargs:
        kw.update(trace_kwargs)
    res = run_bass_kernel_spmd(nc, in_maps, list(range(len(cores))), **kw)
    return [res.results[i]["y"] for i in range(len(cores))], res


def kernel(**inputs):
    n_chunks = B_CORE // N
    ys, _ = run_cores(inputs, n_chunks, list(range(N_CORES)))
    out = np.empty((B_FULL, 1), np.float32)
    for r in range(N_CORES):
        out[r * B_CORE:(r + 1) * B_CORE, 0] = ys[r][0].astype(np.float32)
    return out


# revision 11
# speedup vs baseline: 1.9495x; 1.8980x over previous
"""Trainium2 Bass kernel for nn_Net_67954972557347 (dense_mlp).

Network: a1 = lrelu(a@Wa+ba) [B,68]; b1 = lrelu(b@Wb+bb) [B,68];
c = [a1|b1|meta] [B,140]; then 10 lrelu'd dense layers
(140->34->34->20->20->20->20->20->5->2->1), lrelu slope 0.01.

Strategy: pure data parallel over 8 cores (32768 rows each), activations
feature-major ([feat, batch]), batch streamed 512 columns per chunk.
All tensors fp16 (same 10-bit mantissa as tf32; fp32 PSUM accumulation).

12 layers in SIX matmuls per chunk via a software pipeline (one
layer-hop per step; stages of different chunks colocate). DMA'd tiles
are full 128-partition (partial-partition DMAs run ~10x slower), state
tiles are drain-only:

  TA [a(45); ones; pad->128]   (quad, full-width DMA)
  TB [b(102); ones; ilrelu(meta)(4); pad->128]  (quad, full-width DMA)
  S  [c0(34); c2(20); c6(20); c8(2); c4(20); one(1)]  (per step, drained)
  U[:, 0:512]=TC [a1; c1; c3; c7; y];  U[:, 512:1024]=TD [b1; c5; meta; one]

  G1i: TA[0:46]  -> Qab bank0 (a1+ba)              start
  G1s: S[0:97]   -> Qab bank0 (c1,c3,c7,y + biases) stop
  G2 : TB[0:107] -> Qab bank1 (b1+bb, meta-pass, one-pass) start
  G5 : S[0:97]   -> Qab bank1 (c5+B5)               stop
  G3 : U-TC      -> Q34 (c0p, c2, c4, c8)           start
  G4 : U-TD      -> Q34 (c0p, c6, one-pass, even biases) stop

Drains (2 per chunk): Qab[0:128, 0:1024] -> U' (one ScalarE Prelu over
2 PSUM banks); Q34[0:97] -> S' (VectorE cast + stt lrelu). The ones row
self-sustains: TB ones -> U-TD one -> Q34 col 96 -> S one. meta rides
G2 as identity passthrough (host pre-applies inverse-lrelu). y is DMA'd
from U row 127 on the gpsimd queue.
"""

import os
import sys

import numpy as np

for _p in ("/opt/trn_rl_repo", "/root/.axon_site/_ro/trn_rl_repo"):
    if os.path.isdir(_p) and _p not in sys.path:
        sys.path.append(_p)

import concourse.bass as bass
import concourse.mybir as mybir
import concourse.tile as tile
from concourse import bacc
from concourse.bass_utils import run_bass_kernel_spmd
from bass_rust import add_dep_helper

F16 = mybir.dt.float16
F32 = mybir.dt.float32
ALU = mybir.AluOpType
PRELU = mybir.ActivationFunctionType.Prelu

B_FULL = 262144
N_CORES = 8
B_CORE = B_FULL // N_CORES          # 32768
N = 512                              # columns per chunk (PSUM fp32 cap)
PIPE = 10                            # pipeline depth in steps
ALPHA = 0.01
QUAD = 4                             # chunks per input tile / DMA

# TA rows
TA_A, TA_ONE, TA_END = 0, 45, 46
# TB rows
TB_B, TB_ONE, TB_M, TB_END = 0, 102, 103, 107
# S rows (== Q34 column layout)
S_C0, S_C2, S_C6, S_C8, S_C4, S_ONE, S_END = 0, 34, 54, 74, 76, 96, 97
# U cols 0:512 = TC rows (== Qab bank0 cols)
TC_A1, TC_C1, TC_C3, TC_C7, TC_Y, TC_END = 0, 68, 102, 122, 127, 128
# U cols 512:1024 = TD rows (== Qab bank1 cols)
TD_B1, TD_C5, TD_M, TD_ONE, TD_END = 0, 68, 88, 92, 93


def _pack_weights(inp):
    """Six [128,128] fp16 stationaries packed into one [128, 768] tile."""
    f32 = lambda k: np.asarray(inp[k], np.float32)
    Wa, ba, Wb, bb = f32("Wa"), f32("ba"), f32("Wb"), f32("bb")
    W = [f32(f"W{i}") for i in range(10)]
    B = [f32(f"B{i}") for i in range(10)]
    wt = np.zeros((128, 768), np.float32)

    g1i = wt[:, 0:128]                 # TA -> bank0
    g1i[TA_A:TA_ONE, TC_A1:TC_C1] = Wa
    g1i[TA_ONE, TC_A1:TC_C1] = ba

    g1s = wt[:, 128:256]               # S -> bank0
    g1s[S_C0:S_C2, TC_C1:TC_C3] = W[1]
    g1s[S_C2:S_C6, TC_C3:TC_C7] = W[3]
    g1s[S_C6:S_C8, TC_C7:TC_Y] = W[7]
    g1s[S_C8:S_C4, TC_Y:TC_END] = W[9]
    g1s[S_ONE, TC_C1:TC_C3] = B[1]
    g1s[S_ONE, TC_C3:TC_C7] = B[3]
    g1s[S_ONE, TC_C7:TC_Y] = B[7]
    g1s[S_ONE, TC_Y:TC_END] = B[9]

    g2 = wt[:, 256:384]                # TB -> bank1
    g2[TB_B:TB_ONE, TD_B1:TD_C5] = Wb
    g2[TB_ONE, TD_B1:TD_C5] = bb
    g2[TB_ONE, TD_ONE] = 1.0                      # ones passthrough
    g2[TB_M:TB_END, TD_M:TD_ONE] = np.eye(4)      # meta passthrough

    g5 = wt[:, 384:512]                # S -> bank1
    g5[S_C4:S_ONE, TD_C5:TD_M] = W[5]
    g5[S_ONE, TD_C5:TD_M] = B[5]

    g3 = wt[:, 512:640]                # U-TC -> Q34
    g3[TC_A1:TC_C1, S_C0:S_C2] = W[0][0:68]
    g3[TC_C1:TC_C3, S_C2:S_C6] = W[2]
    g3[TC_C3:TC_C7, S_C4:S_ONE] = W[4]
    g3[TC_C7:TC_Y, S_C8:S_C4] = W[8]

    g4 = wt[:, 640:768]                # U-TD -> Q34
    g4[TD_B1:TD_C5, S_C0:S_C2] = W[0][68:136]
    g4[TD_C5:TD_M, S_C6:S_C8] = W[6]
    g4[TD_M:TD_ONE, S_C0:S_C2] = W[0][136:140]
    g4[TD_ONE, S_C0:S_C2] = B[0]
    g4[TD_ONE, S_C2:S_C6] = B[2]
    g4[TD_ONE, S_C6:S_C8] = B[6]
    g4[TD_ONE, S_C8:S_C4] = B[8]
    g4[TD_ONE, S_C4:S_ONE] = B[4]
    g4[TD_ONE, S_ONE] = 1.0                       # ones passthrough

    return wt.astype(np.float16)


def _pack_core_inputs(a, b, meta, n_stream):
    """One core's shard -> (tA [128, ns*N], tB [128, ns*N]) fp16.

    Full 128-partition streams (pad rows zero); columns past the shard
    replicate column 0 so tail-step quad DMAs read valid data."""
    bc = a.shape[0]
    ns = n_stream * N
    tA = np.zeros((128, ns), np.float16)
    tA[TA_A:TA_ONE, :bc] = a.astype(np.float16).T
    tA[TA_ONE] = 1.0
    tB = np.zeros((128, ns), np.float16)
    tB[TB_B:TB_ONE, :bc] = b.astype(np.float16).T
    tB[TB_ONE] = 1.0
    m = meta.astype(np.float32)
    tB[TB_M:TB_END, :bc] = np.where(m > 0, m, m * 100.0).astype(np.float16).T
    tA[TA_A:TA_ONE, bc:] = tA[TA_A:TA_ONE, 0:1]
    tB[TB_B:TB_ONE, bc:] = tB[TB_B:TB_ONE, 0:1]
    tB[TB_M:TB_END, bc:] = tB[TB_M:TB_END, 0:1]
    return tA, tB


def build_bass(n_chunks):
    nc = bacc.Bacc(None, target_bir_lowering=False, debug=False)
    n_steps = n_chunks + PIPE                       # 74
    n_quads = (n_steps + 1 + QUAD - 1) // QUAD      # tiles 0..n_steps
    n_stream = n_quads * QUAD

    tA_d = nc.dram_tensor("tA", [128, n_stream * N], F16, kind="ExternalInput")
    tB_d = nc.dram_tensor("tB", [128, n_stream * N], F16, kind="ExternalInput")
    wt_d = nc.dram_tensor("wt", [128, 768], F16, kind="ExternalInput")
    y_d = nc.dram_tensor("y", [1, n_chunks * N], F16, kind="ExternalOutput")

    with tile.TileContext(nc) as tc:
        with (
            tc.tile_pool(name="const", bufs=1) as constp,
            tc.tile_pool(name="tap", bufs=3) as tap,
            tc.tile_pool(name="tbp", bufs=3) as tbp,
            tc.tile_pool(name="sp", bufs=3) as spool,
            tc.tile_pool(name="up", bufs=3) as up,
            tc.tile_pool(name="ps", bufs=2, space=bass.MemorySpace.PSUM) as ps,
        ):
            wt = constp.tile([128, 768], F16, tag="wt")
            nc.sync.dma_start(wt[:], wt_d[:])
            wg1i, wg1s, wg2 = wt[:, 0:128], wt[:, 128:256], wt[:, 256:384]
            wg5, wg3, wg4 = wt[:, 384:512], wt[:, 512:640], wt[:, 640:768]

            def chain(*insts):
                for i in range(1, len(insts)):
                    add_dep_helper(insts[i].ins, insts[i - 1].ins,
                                   sync=False, reason="psum acc order")

            TAq, TBq, S, U = {}, {}, {}, {}

            def new_quad(p):
                TAq[p] = tap.tile([128, QUAD * N], F16, tag="ta", name=f"ta_{p}")
                TBq[p] = tbp.tile([128, QUAD * N], F16, tag="tb", name=f"tb_{p}")
                qs = slice(p * QUAD * N, (p + 1) * QUAD * N)
                nc.sync.dma_start(TAq[p][0:128], tA_d[:, qs])
                nc.sync.dma_start(TBq[p][0:128], tB_d[:, qs])

            def new_state(t):
                S[t] = spool.tile([128, N], F16, tag="s", name=f"s_{t}")
                U[t] = up.tile([128, 2 * N], F16, tag="u", name=f"u_{t}")
                if t == 0:
                    nc.gpsimd.memset(S[t][0:128], 0.0)
                    nc.gpsimd.memset(U[t][0:128], 0.0)

            new_quad(0)
            new_state(0)

            mm = nc.tensor.matmul
            for t in range(n_steps):
                if (t + 1) % QUAD == 0:
                    new_quad((t + 1) // QUAD)
                new_state(t + 1)
                ta = TAq[t // QUAD]
                tb = TBq[t // QUAD]
                cs = slice((t % QUAD) * N, (t % QUAD + 1) * N)

                qab = ps.tile([128, 2 * N], F32, tag="qab", name=f"qab_{t}")
                i1 = mm(qab[0:128, 0:N], wg1i[0:TA_END], ta[0:TA_END, cs],
                        start=True, stop=False, tile_position=(0, 0))
                i2 = mm(qab[0:128, 0:N], wg1s[0:S_END], S[t][0:S_END],
                        start=False, stop=True, tile_position=(0, 0))
                chain(i1, i2)
                i3 = mm(qab[0:128, N:2 * N], wg2[0:TB_END], tb[0:TB_END, cs],
                        start=True, stop=False, tile_position=(0, 0))
                i4 = mm(qab[0:128, N:2 * N], wg5[0:S_END], S[t][0:S_END],
                        start=False, stop=True, tile_position=(0, 0))
                chain(i3, i4)
                q34 = ps.tile([128, N], F32, tag="q34", name=f"q34_{t}")
                i5 = mm(q34[0:128], wg3[0:TC_END], U[t][0:TC_END, 0:N],
                        start=True, stop=False, tile_position=(0, 0))
                i6 = mm(q34[0:128], wg4[0:TD_END], U[t][0:TD_END, N:2 * N],
                        start=False, stop=True, tile_position=(0, 0))
                chain(i5, i6)

                # ---- drains into step t+1 tiles ----
                nc.scalar.activation(U[t + 1][0:128, 0:2 * N],
                                     qab[0:128, 0:2 * N], PRELU, alpha=ALPHA)
                nc.vector.tensor_copy(S[t + 1][0:S_END], q34[0:S_END])
                nc.vector.scalar_tensor_tensor(
                    S[t + 1][0:S_END], S[t + 1][0:S_END],
                    ALPHA, S[t + 1][0:S_END], ALU.mult, ALU.max)

                # ---- y out (chunk t-10 sits in U[t+1] row 127, TC half) ----
                if t >= PIPE:
                    c = t - PIPE
                    nc.gpsimd.dma_start(y_d[:, c * N:(c + 1) * N],
                                        U[t + 1][TC_Y:TC_END, 0:N])

    nc.compile()
    return nc


_NC_CACHE = {}


def _get_nc(n_chunks):
    if n_chunks not in _NC_CACHE:
        _NC_CACHE[n_chunks] = build_bass(n_chunks)
    return _NC_CACHE[n_chunks]


def run_cores(inputs, n_chunks, cores, trace=False, trace_kwargs=None):
    a = np.asarray(inputs["a"], np.float32)
    b = np.asarray(inputs["b"], np.float32)
    meta = np.asarray(inputs["meta"], np.float32)
    wt = _pack_weights(inputs)
    n_steps = n_chunks + PIPE
    n_stream = ((n_steps + 1 + QUAD - 1) // QUAD) * QUAD
    in_maps = []
    for r in cores:
        sl = slice(r * B_CORE, r * B_CORE + n_chunks * N)
        tA, tB = _pack_core_inputs(a[sl], b[sl], meta[sl], n_stream)
        in_maps.append({"tA": tA, "tB": tB, "wt": wt})
    nc = _get_nc(n_chunks)
    kw = dict(trace=trace)
    if trace_kwargs:
        kw.update(trace_kwargs)
    res = run_bass_kernel_spmd(nc, in_maps, list(range(len(cores))), **kw)
    return [res.results[i]["y"] for i in range(len(cores))], res


def kernel(**inputs):
    n_chunks = B_CORE // N
    ys, _ = run_cores(inputs, n_chunks, list(range(N_CORES)))
    out = np.empty((B_FULL, 1), np.float32)
    for r in range(N_CORES):
        out[r * B_CORE:(r + 1) * B_CORE, 0] = ys[r][0].astype(np.float32)
    return out
